# revision 7
# baseline (speedup 1.0000x reference)
"""AttentionBlock (GroupNorm -> 1x1 qkv -> 4-head attention -> 1x1 proj -> residual)
on 8 trn2 NeuronCores, data-parallel over the batch dim (B=8, one element/core).

Layout per core: channel-major [C=512, N=1024] as 4 SBUF tiles of [128, 1024].
All matmuls in float32r (1 cycle/row for free dim >= 256). V is computed
spatial-major directly from the qkv matmul so attention needs no transposes:
  ST[j,i] = sum_d k[d,j] q[d,i]   (K-tile stationary)
  p~T[j,i] = exp(scale*ST)        (ScalarE, PSUM->SBUF)
  rowsum[i] = ones^T @ p~T        (PE)
  PV[d,i] = sum_j v_sp[j,d] p~T[j,i]   -> channel-major attention output
  out = PV * (1/rowsum broadcast)      (softmax divide deferred past PV)
v-bias is folded into proj bias on the host (softmax rows sum to 1).
"""

import numpy as np

B, C, H, W = 8, 512, 32, 32
N = H * W  # 1024
NUM_HEADS = 4
HEAD_DIM = C // NUM_HEADS  # 128
NUM_GROUPS = 32
GROUP_CH = C // NUM_GROUPS  # 16
EPS = 1e-5
NT = C // 128  # 4 channel tiles
NO_QK = 8  # q,k output tiles (1024 channels)
SCALE = 1.0 / float(np.sqrt(HEAD_DIM))
N_CORES = 8


def build_bass():
    import concourse.bacc as bacc
    import concourse.tile as tile
    from concourse import mybir

    f32 = mybir.dt.float32
    f32r = mybir.dt.float32r
    Act = mybir.ActivationFunctionType
    Alu = mybir.AluOpType
    Ax = mybir.AxisListType

    nc = bacc.Bacc("TRN2", target_bir_lowering=False, debug=False,
                   num_devices=N_CORES)

    d_x = nc.declare_dram_parameter("x", [C, N], f32, isOutput=False)
    d_wt = nc.declare_dram_parameter("qkv_wt", [C, 3 * C], f32r, isOutput=False)
    d_pwt = nc.declare_dram_parameter("proj_wt", [C, C], f32r, isOutput=False)
    d_bqk = nc.declare_dram_parameter("bias_qk", [2 * C], f32, isOutput=False)
    d_beff = nc.declare_dram_parameter("b_eff", [C], f32, isOutput=False)
    d_gam = nc.declare_dram_parameter("norm_w", [C], f32, isOutput=False)
    d_bet = nc.declare_dram_parameter("norm_b", [C], f32, isOutput=False)
    d_sel = nc.declare_dram_parameter("sel", [C, NUM_GROUPS], f32, isOutput=False)
    d_ones = nc.declare_dram_parameter("ones", [128, 1], f32r, isOutput=False)
    d_warm = nc.declare_dram_parameter("warm", [128, 512], f32r, isOutput=False)
    d_selT = nc.declare_dram_parameter("selT", [NUM_GROUPS, C], f32, isOutput=False)
    d_out = nc.declare_dram_parameter("out", [C, N], f32, isOutput=True)

    def r(ap):
        return ap.bitcast(f32r)

    with tile.TileContext(nc) as tc:
        with (
            tc.tile_pool(name="persist", bufs=1) as pp,
            tc.tile_pool(name="pt", bufs=8) as p_pt,
            tc.tile_pool(name="outp", bufs=2) as p_out,
            tc.tile_pool(name="small", bufs=1) as ps,
            tc.tile_pool(name="psum", bufs=4, space="PSUM") as pm,
        ):
            # ---- constant / weight loads ----
            # small consts first (scalar HWDGE queue) so GN isn't blocked
            warm = ps.tile([128, 512], f32r, tag="warm", name="warm")
            nc.scalar.dma_start(warm[:], d_warm[:, :])
            sels, selTs = [], []
            for t in range(NT):
                sel_t = ps.tile([128, NUM_GROUPS], f32, tag=f"sel{t}", name=f"sel{t}")
                nc.scalar.dma_start(sel_t[:], d_sel[t * 128:(t + 1) * 128, :])
                sels.append(sel_t)
                selT_t = ps.tile([NUM_GROUPS, 128], f32, tag=f"selT{t}", name=f"selT{t}")
                nc.scalar.dma_start(selT_t[:], d_selT[:, t * 128:(t + 1) * 128])
                selTs.append(selT_t)
            gam = ps.tile([128, NT], f32, tag="gam", name="gam")
            nc.scalar.dma_start(gam[:], d_gam.rearrange("(a p) -> p a", p=128))
            bet = ps.tile([128, NT], f32, tag="bet", name="bet")
            nc.scalar.dma_start(bet[:], d_bet.rearrange("(a p) -> p a", p=128))
            bqk = ps.tile([128, NO_QK], f32, tag="bqk", name="bqk")
            nc.scalar.dma_start(bqk[:], d_bqk.rearrange("(a p) -> p a", p=128))
            beff = ps.tile([128, NT], f32, tag="beff", name="beff")
            nc.scalar.dma_start(beff[:], d_beff.rearrange("(a p) -> p a", p=128))
            ones128 = ps.tile([128, 1], f32r, tag="ones128", name="ones128")
            nc.scalar.dma_start(ones128[:], d_ones[:, :])
            epsv = ps.tile([NUM_GROUPS, 1], f32, tag="epsv", name="epsv")
            nc.vector.memset(epsv[:], EPS)
            # x on sync queue; qkv weights split across both queues
            xs, hs, wts, pwts = [], [], [], []
            for t in range(NT):
                x_t = pp.tile([128, N], f32, tag=f"x{t}", name=f"x{t}")
                nc.sync.dma_start(x_t[:], d_x[t * 128:(t + 1) * 128, :])
                xs.append(x_t)
            for t in range(NT):
                wt_t = pp.tile([128, 3 * C], f32r, tag=f"wt{t}", name=f"wt{t}")
                eng = nc.scalar if t % 2 == 1 else nc.sync
                eng.dma_start(wt_t[:], d_wt[t * 128:(t + 1) * 128, :])
                wts.append(wt_t)
            for t in range(NT):
                pwt_t = pp.tile([128, C], f32r, tag=f"pwt{t}", name=f"pwt{t}")
                nc.scalar.dma_start(pwt_t[:], d_pwt[t * 128:(t + 1) * 128, :])
                pwts.append(pwt_t)

            # PE warm-up chain: keeps the HAM clock-gate open while weights
            # stream in; result never read
            junk = pm.tile([128, N], f32, tag="ps", name="junk")
            NJ = 30
            for j in range(NJ):
                nc.tensor.matmul(junk[0:128, 0:512], warm[:, 0:128],
                                 warm[:, 0:512],
                                 start=(j == 0), stop=(j == NJ - 1))

            # ---- group norm: stats ----
            stats = []
            for t in range(NT):
                h_t = pp.tile([128, N], f32r, tag=f"h{t}", name=f"h{t}")
                hs.append(h_t)
                st_t = ps.tile([128, 2], f32, tag=f"st{t}", name=f"st{t}")
                nc.vector.reduce_sum(st_t[:, 0:1], xs[t][:], axis=Ax.X)
                # square via ACT, accumulate sum(x^2); main out is scratch (h_t,
                # overwritten by the normalize pass below)
                nc.scalar.activation(h_t[:], xs[t][:], Act.Square,
                                     accum_out=st_t[:, 1:2])
                stats.append(st_t)

            psg = pm.tile([128, N], f32, tag="ps", name="psg")
            for t in range(NT):
                nc.tensor.matmul(psg[0:NUM_GROUPS, 0:2], sels[t][:],
                                 stats[t][:, 0:2],
                                 start=(t == 0), stop=(t == NT - 1))
            # per-group: mean, rstd  (cols of msr: 0=mean 1=rstd 2=var 3=tmp)
            inv_n = 1.0 / float(GROUP_CH * N)
            msr = ps.tile([NUM_GROUPS, 4], f32, tag="msr", name="msr")
            nc.scalar.mul(msr[:, 0:1], psg[0:NUM_GROUPS, 0:1], inv_n)
            nc.scalar.square(msr[:, 3:4], msr[:, 0:1])
            nc.vector.scalar_tensor_tensor(msr[:, 2:3], psg[0:NUM_GROUPS, 1:2],
                                           inv_n, msr[:, 3:4],
                                           op0=Alu.mult, op1=Alu.subtract)
            nc.scalar.activation(msr[:, 3:4], msr[:, 2:3], Act.Sqrt,
                                 bias=epsv[:, 0:1])
            nc.vector.reciprocal(msr[:, 1:2], msr[:, 3:4])

            # expand to per-channel a,b then h = a*x + b
            abts = []
            for t in range(NT):
                pse = pm.tile([128, N], f32, tag="ps", name=f"pse{t}")
                nc.tensor.matmul(pse[:, 0:2], selTs[t][:], msr[:, 0:2],
                                 start=True, stop=True)
                ab_t = ps.tile([128, 3], f32, tag=f"ab{t}", name=f"ab{t}")
                nc.vector.tensor_mul(ab_t[:, 0:1], gam[:, t:t + 1], pse[:, 1:2])
                nc.vector.tensor_mul(ab_t[:, 2:3], pse[:, 0:1], ab_t[:, 0:1])
                nc.vector.tensor_sub(ab_t[:, 1:2], bet[:, t:t + 1], ab_t[:, 2:3])
                nc.scalar.activation(hs[t][:], xs[t][:], Act.Identity,
                                     bias=ab_t[:, 1:2], scale=ab_t[:, 0:1])
                abts.append(ab_t)

            # ---- qkv: q,k channel-major [1024 ch, N] ----
            qks = []
            for ot in range(NO_QK):
                pq = pm.tile([128, N], f32, tag="ps", name=f"pq{ot}")
                for t in range(NT):
                    for half in range(2):
                        nc.tensor.matmul(
                            pq[:, half * 512:(half + 1) * 512],
                            wts[t][:, ot * 128:(ot + 1) * 128],
                            hs[t][:, half * 512:(half + 1) * 512],
                            start=(t == 0), stop=(t == NT - 1))
                qk_t = pp.tile([128, N], f32r, tag=f"qk{ot}", name=f"qk{ot}")
                nc.vector.tensor_scalar_add(qk_t[:], pq[:], bqk[:, ot:ot + 1])
                qks.append(qk_t)

            # ---- v spatial-major [N, 512] ----
            vs = []
            for nt in range(NO_QK):
                pv_ = pm.tile([128, N], f32, tag="ps", name=f"pvv{nt}")
                for t in range(NT):
                    nc.tensor.matmul(
                        pv_[:, 0:512],
                        hs[t][:, nt * 128:(nt + 1) * 128],
                        wts[t][:, 2 * C:3 * C],
                        start=(t == 0), stop=(t == NT - 1))
                v_t = pp.tile([128, 512], f32r, tag=f"v{nt}", name=f"v{nt}")
                nc.vector.tensor_copy(v_t[:], pv_[:, 0:512])
                vs.append(v_t)

            # ---- attention, head by head ----
            attns = []
            for h in range(NUM_HEADS):
                qT = qks[h]          # [128 d, N]
                kT = qks[NUM_HEADS + h]
                # ST pass + exp
                pts = []
                for jt in range(NO_QK):
                    pst = pm.tile([128, N], f32, tag="ps", name=f"pst{h}_{jt}")
                    for half in range(2):
                        nc.tensor.matmul(
                            pst[:, half * 512:(half + 1) * 512],
                            kT[:, jt * 128:(jt + 1) * 128],
                            qT[:, half * 512:(half + 1) * 512],
                            start=True, stop=True)
                    pt_jt = p_pt.tile([128, N], f32r, tag="pt", name=f"pt{h}_{jt}")
                    nc.scalar.activation(pt_jt[:], pst[:], Act.Exp, scale=SCALE)
                    pts.append(pt_jt)
                # rowsum pass
                prs = pm.tile([128, N], f32, tag="ps", name=f"prs{h}")
                for half in range(2):
                    for jt in range(NO_QK):
                        nc.tensor.matmul(
                            prs[0:1, half * 512:(half + 1) * 512],
                            ones128[:],
                            pts[jt][:, half * 512:(half + 1) * 512],
                            start=(jt == 0), stop=(jt == NO_QK - 1))
                rr = ps.tile([1, N], f32, tag="rr", bufs=2, name=f"rr{h}")
                nc.vector.reciprocal_approx_fast(rr[:], prs[0:1, :])
                # PV pass
                ppv = pm.tile([128, N], f32, tag="ps", name=f"ppv{h}")
                for half in range(2):
                    for jt in range(NO_QK):
                        nc.tensor.matmul(
                            ppv[:, half * 512:(half + 1) * 512],
                            vs[jt][:, h * 128:(h + 1) * 128],
                            pts[jt][:, half * 512:(half + 1) * 512],
                            start=(jt == 0), stop=(jt == NO_QK - 1))
                # broadcast 1/rowsum to all partitions (DMA), then scale
                rb = ps.tile([128, N], f32, tag="rb", bufs=2, name=f"rb{h}")
                nc.gpsimd.partition_broadcast(rb[:], rr[:])
                attn_h = pp.tile([128, N], f32r, tag=f"attn{h}", name=f"attn{h}")
                nc.vector.tensor_mul(attn_h[:], ppv[:], rb[:])
                attns.append(attn_h)

            # ---- proj + bias + residual ----
            for ot in range(NT):
                ppr = pm.tile([128, N], f32, tag="ps", name=f"ppr{ot}")
                for h in range(NUM_HEADS):
                    for half in range(2):
                        nc.tensor.matmul(
                            ppr[:, half * 512:(half + 1) * 512],
                            pwts[h][:, ot * 128:(ot + 1) * 128],
                            attns[h][:, half * 512:(half + 1) * 512],
                            start=(h == 0), stop=(h == NUM_HEADS - 1))
                o_t = p_out.tile([128, N], f32, tag="out", name=f"o{ot}")
                nc.vector.scalar_tensor_tensor(o_t[:], ppr[:],
                                               beff[:, ot:ot + 1], xs[ot][:],
                                               op0=Alu.add, op1=Alu.add)
                eng = nc.scalar if ot % 2 == 1 else nc.sync
                eng.dma_start(d_out[ot * 128:(ot + 1) * 128, :], o_t[:])

    nc.compile()
    return nc


def make_in_maps(x, norm_w, norm_b, qkv_w, qkv_b, proj_w, proj_b):
    x = np.asarray(x, dtype=np.float32)
    qkv_w = np.asarray(qkv_w, dtype=np.float32)
    qkv_b = np.asarray(qkv_b, dtype=np.float32)
    proj_w = np.asarray(proj_w, dtype=np.float32)
    proj_b = np.asarray(proj_b, dtype=np.float32)

    wt = np.ascontiguousarray(qkv_w.T)                      # [C, 3C]
    pwt = np.ascontiguousarray(proj_w.T)                    # [C, C]
    b_eff = (proj_b + proj_w @ qkv_b[2 * C:3 * C]).astype(np.float32)
    bias_qk = np.ascontiguousarray(qkv_b[:2 * C])

    ch = np.arange(C)
    sel = (ch[:, None] // GROUP_CH == np.arange(NUM_GROUPS)[None, :]).astype(np.float32)
    selT = np.ascontiguousarray(sel.T)

    xs = x.reshape(B, C, N)
    common = {
        "qkv_wt": wt, "proj_wt": pwt, "bias_qk": bias_qk, "b_eff": b_eff,
        "norm_w": np.ascontiguousarray(norm_w, dtype=np.float32),
        "norm_b": np.ascontiguousarray(norm_b, dtype=np.float32),
        "sel": sel, "selT": selT, "ones": np.ones((128, 1), np.float32),
        "warm": np.full((128, 512), 0.5, np.float32),
    }
    return [dict(common, x=np.ascontiguousarray(xs[i])) for i in range(B)]


def run(inputs, trace=False, tmpdir=None):
    from concourse.bass_utils import run_bass_kernel_spmd
    nc = build_bass()
    in_maps = make_in_maps(**inputs)
    res = run_bass_kernel_spmd(nc, in_maps, core_ids=list(range(N_CORES)),
                               trace=trace, tmpdir=tmpdir)
    out = np.stack([res.results[i]["out"] for i in range(N_CORES)])
    return out.reshape(B, C, H, W).astype(np.float32), res


def kernel(**inputs):
    out, _ = run(inputs, trace=False)
    return out


# revision 8
# speedup vs baseline: 1.0702x; 1.0702x over previous
"""AttentionBlock (GroupNorm -> 1x1 qkv -> 4-head attention -> 1x1 proj -> residual)
on 8 trn2 NeuronCores, data-parallel over the batch dim (B=8, one element/core).

Layout per core: channel-major [C=512, N=1024] as 4 SBUF tiles of [128, 1024].
All matmuls in float32r (1 cycle/row for free dim >= 256). V is computed
spatial-major directly from the qkv matmul so attention needs no transposes:
  ST[j,i] = sum_d k[d,j] q[d,i]   (K-tile stationary)
  p~T[j,i] = exp(scale*ST)        (ScalarE, PSUM->SBUF)
  rowsum[i] = ones^T @ p~T        (PE)
  PV[d,i] = sum_j v_sp[j,d] p~T[j,i]   -> channel-major attention output
  out = PV * (1/rowsum broadcast)      (softmax divide deferred past PV)
v-bias is folded into proj bias on the host (softmax rows sum to 1).
"""

import numpy as np

B, C, H, W = 8, 512, 32, 32
N = H * W  # 1024
NUM_HEADS = 4
HEAD_DIM = C // NUM_HEADS  # 128
NUM_GROUPS = 32
GROUP_CH = C // NUM_GROUPS  # 16
EPS = 1e-5
NT = C // 128  # 4 channel tiles
NO_QK = 8  # q,k output tiles (1024 channels)
SCALE = 1.0 / float(np.sqrt(HEAD_DIM))
N_CORES = 8


def build_bass():
    import concourse.bacc as bacc
    import concourse.tile as tile
    from concourse import mybir

    f32 = mybir.dt.float32
    bf16 = mybir.dt.bfloat16
    Act = mybir.ActivationFunctionType
    Alu = mybir.AluOpType
    Ax = mybir.AxisListType

    nc = bacc.Bacc("TRN2", target_bir_lowering=False, debug=False,
                   num_devices=N_CORES)

    d_x = nc.declare_dram_parameter("x", [C, N], f32, isOutput=False)
    d_wt = nc.declare_dram_parameter("qkv_wt", [C, 3 * C], bf16, isOutput=False)
    d_pwt = nc.declare_dram_parameter("proj_wt", [C, C], bf16, isOutput=False)
    d_bqk = nc.declare_dram_parameter("bias_qk", [2 * C], f32, isOutput=False)
    d_beff = nc.declare_dram_parameter("b_eff", [C], f32, isOutput=False)
    d_gam = nc.declare_dram_parameter("norm_w", [C], f32, isOutput=False)
    d_bet = nc.declare_dram_parameter("norm_b", [C], f32, isOutput=False)
    d_sel = nc.declare_dram_parameter("sel", [C, NUM_GROUPS], f32, isOutput=False)
    d_warm = nc.declare_dram_parameter("warm", [128, 512], bf16, isOutput=False)
    d_selT = nc.declare_dram_parameter("selT", [NUM_GROUPS, C], f32, isOutput=False)
    d_out = nc.declare_dram_parameter("out", [C, N], f32, isOutput=True)

    with tile.TileContext(nc) as tc:
        with (
            tc.tile_pool(name="persist", bufs=1) as pp,
            tc.tile_pool(name="pt", bufs=8) as p_pt,
            tc.tile_pool(name="outp", bufs=2) as p_out,
            tc.tile_pool(name="small", bufs=1) as ps,
            tc.tile_pool(name="psum", bufs=4, space="PSUM") as pm,
        ):
            # ---- constant / weight loads ----
            # one queue, strict priority order: consts, x, wt, pwt
            warm = ps.tile([128, 512], bf16, tag="warm", name="warm")
            nc.sync.dma_start(warm[:], d_warm[:, :])
            sels, selTs = [], []
            for t in range(NT):
                sel_t = ps.tile([128, NUM_GROUPS], f32, tag=f"sel{t}", name=f"sel{t}")
                nc.sync.dma_start(sel_t[:], d_sel[t * 128:(t + 1) * 128, :])
                sels.append(sel_t)
                selT_t = ps.tile([NUM_GROUPS, 128], f32, tag=f"selT{t}", name=f"selT{t}")
                nc.sync.dma_start(selT_t[:], d_selT[:, t * 128:(t + 1) * 128])
                selTs.append(selT_t)
            gam = ps.tile([128, NT], f32, tag="gam", name="gam")
            nc.sync.dma_start(gam[:], d_gam.rearrange("(a p) -> p a", p=128))
            bet = ps.tile([128, NT], f32, tag="bet", name="bet")
            nc.sync.dma_start(bet[:], d_bet.rearrange("(a p) -> p a", p=128))
            bqk = ps.tile([128, NO_QK], f32, tag="bqk", name="bqk")
            nc.sync.dma_start(bqk[:], d_bqk.rearrange("(a p) -> p a", p=128))
            beff = ps.tile([128, NT], f32, tag="beff", name="beff")
            nc.sync.dma_start(beff[:], d_beff.rearrange("(a p) -> p a", p=128))
            ones128 = ps.tile([128, 1], bf16, tag="ones128", name="ones128")
            nc.vector.memset(ones128[:], 1.0)
            epsv = ps.tile([NUM_GROUPS, 1], f32, tag="epsv", name="epsv")
            nc.vector.memset(epsv[:], EPS)
            xs, hs, wts, pwts = [], [], [], []
            for t in range(NT):
                x_t = pp.tile([128, N], f32, tag=f"x{t}", name=f"x{t}")
                nc.sync.dma_start(x_t[:], d_x[t * 128:(t + 1) * 128, :])
                xs.append(x_t)
            for t in range(NT):
                wt_t = pp.tile([128, 3 * C], bf16, tag=f"wt{t}", name=f"wt{t}")
                nc.sync.dma_start(wt_t[:], d_wt[t * 128:(t + 1) * 128, :])
                wts.append(wt_t)
            for t in range(NT):
                pwt_t = pp.tile([128, C], bf16, tag=f"pwt{t}", name=f"pwt{t}")
                nc.sync.dma_start(pwt_t[:], d_pwt[t * 128:(t + 1) * 128, :])
                pwts.append(pwt_t)

            # PE warm-up chain: keeps the HAM clock-gate open while inputs
            # stream in; result never read
            junk = pm.tile([128, N], f32, tag="ps", name="junk")
            NJ = 40
            for j in range(NJ):
                nc.tensor.matmul(junk[0:128, 0:512], warm[:, 0:128],
                                 warm[:, 0:512],
                                 start=(j == 0), stop=(j == NJ - 1))

            # ---- group norm: stats ----
            stats = []
            for t in range(NT):
                h_t = pp.tile([128, N], bf16, tag=f"h{t}", name=f"h{t}")
                hs.append(h_t)
                st_t = ps.tile([128, 2], f32, tag=f"st{t}", name=f"st{t}")
                nc.vector.reduce_sum(st_t[:, 0:1], xs[t][:], axis=Ax.X)
                # square via ACT, accumulate sum(x^2); main out is scratch (h_t,
                # overwritten by the normalize pass below)
                nc.scalar.activation(h_t[:], xs[t][:], Act.Square,
                                     accum_out=st_t[:, 1:2])
                stats.append(st_t)

            psg = pm.tile([128, N], f32, tag="ps", name="psg")
            for t in range(NT):
                nc.tensor.matmul(psg[0:NUM_GROUPS, 0:2], sels[t][:],
                                 stats[t][:, 0:2],
                                 start=(t == 0), stop=(t == NT - 1))
            # per-group: mean, rstd  (cols of msr: 0=mean 1=rstd 2=var 3=tmp)
            inv_n = 1.0 / float(GROUP_CH * N)
            msr = ps.tile([NUM_GROUPS, 4], f32, tag="msr", name="msr")
            nc.scalar.mul(msr[:, 0:1], psg[0:NUM_GROUPS, 0:1], inv_n)
            nc.scalar.square(msr[:, 3:4], msr[:, 0:1])
            nc.vector.scalar_tensor_tensor(msr[:, 2:3], psg[0:NUM_GROUPS, 1:2],
                                           inv_n, msr[:, 3:4],
                                           op0=Alu.mult, op1=Alu.subtract)
            nc.scalar.activation(msr[:, 3:4], msr[:, 2:3], Act.Sqrt,
                                 bias=epsv[:, 0:1])
            nc.vector.reciprocal(msr[:, 1:2], msr[:, 3:4])

            # expand to per-channel a,b then h = a*x + b
            abts = []
            for t in range(NT):
                pse = pm.tile([128, N], f32, tag="ps", name=f"pse{t}")
                nc.tensor.matmul(pse[:, 0:2], selTs[t][:], msr[:, 0:2],
                                 start=True, stop=True)
                ab_t = ps.tile([128, 3], f32, tag=f"ab{t}", name=f"ab{t}")
                nc.vector.tensor_mul(ab_t[:, 0:1], gam[:, t:t + 1], pse[:, 1:2])
                nc.vector.tensor_mul(ab_t[:, 2:3], pse[:, 0:1], ab_t[:, 0:1])
                nc.vector.tensor_sub(ab_t[:, 1:2], bet[:, t:t + 1], ab_t[:, 2:3])
                nc.scalar.activation(hs[t][:], xs[t][:], Act.Identity,
                                     bias=ab_t[:, 1:2], scale=ab_t[:, 0:1])
                abts.append(ab_t)

            # ---- qkv: q,k channel-major [1024 ch, N] ----
            qks = []
            for ot in range(NO_QK):
                pq = pm.tile([128, N], f32, tag="ps", name=f"pq{ot}")
                for t in range(NT):
                    for half in range(2):
                        nc.tensor.matmul(
                            pq[:, half * 512:(half + 1) * 512],
                            wts[t][:, ot * 128:(ot + 1) * 128],
                            hs[t][:, half * 512:(half + 1) * 512],
                            start=(t == 0), stop=(t == NT - 1))
                qk_t = pp.tile([128, N], bf16, tag=f"qk{ot}", name=f"qk{ot}")
                nc.vector.tensor_scalar_add(qk_t[:], pq[:], bqk[:, ot:ot + 1])
                qks.append(qk_t)

            # ---- v spatial-major [N, 512] ----
            vs = []
            for nt in range(NO_QK):
                pv_ = pm.tile([128, N], f32, tag="ps", name=f"pvv{nt}")
                for t in range(NT):
                    nc.tensor.matmul(
                        pv_[:, 0:512],
                        hs[t][:, nt * 128:(nt + 1) * 128],
                        wts[t][:, 2 * C:3 * C],
                        start=(t == 0), stop=(t == NT - 1))
                v_t = pp.tile([128, 512], bf16, tag=f"v{nt}", name=f"v{nt}")
                nc.vector.tensor_copy(v_t[:], pv_[:, 0:512])
                vs.append(v_t)

            # ---- attention, head by head ----
            attns = []
            for h in range(NUM_HEADS):
                qT = qks[h]          # [128 d, N]
                kT = qks[NUM_HEADS + h]
                # ST pass + exp
                pts = []
                for jt in range(NO_QK):
                    pst = pm.tile([128, N], f32, tag="ps", name=f"pst{h}_{jt}")
                    for half in range(2):
                        nc.tensor.matmul(
                            pst[:, half * 512:(half + 1) * 512],
                            kT[:, jt * 128:(jt + 1) * 128],
                            qT[:, half * 512:(half + 1) * 512],
                            start=True, stop=True)
                    pt_jt = p_pt.tile([128, N], bf16, tag="pt", name=f"pt{h}_{jt}")
                    nc.scalar.activation(pt_jt[:], pst[:], Act.Exp, scale=SCALE)
                    pts.append(pt_jt)
                # rowsum pass
                prs = pm.tile([128, N], f32, tag="ps", name=f"prs{h}")
                for half in range(2):
                    for jt in range(NO_QK):
                        nc.tensor.matmul(
                            prs[0:1, half * 512:(half + 1) * 512],
                            ones128[:],
                            pts[jt][:, half * 512:(half + 1) * 512],
                            start=(jt == 0), stop=(jt == NO_QK - 1))
                rr = ps.tile([1, N], f32, tag="rr", bufs=2, name=f"rr{h}")
                nc.vector.reciprocal_approx_fast(rr[:], prs[0:1, :])
                # PV pass
                ppv = pm.tile([128, N], f32, tag="ps", name=f"ppv{h}")
                for half in range(2):
                    for jt in range(NO_QK):
                        nc.tensor.matmul(
                            ppv[:, half * 512:(half + 1) * 512],
                            vs[jt][:, h * 128:(h + 1) * 128],
                            pts[jt][:, half * 512:(half + 1) * 512],
                            start=(jt == 0), stop=(jt == NO_QK - 1))
                # broadcast 1/rowsum to all partitions (DMA), then scale
                rb = ps.tile([128, N], f32, tag="rb", bufs=2, name=f"rb{h}")
                nc.gpsimd.partition_broadcast(rb[:], rr[:])
                attn_h = pp.tile([128, N], bf16, tag=f"attn{h}", name=f"attn{h}")
                nc.vector.tensor_mul(attn_h[:], ppv[:], rb[:])
                attns.append(attn_h)

            # ---- proj + bias + residual ----
            for ot in range(NT):
                ppr = pm.tile([128, N], f32, tag="ps", name=f"ppr{ot}")
                for h in range(NUM_HEADS):
                    for half in range(2):
                        nc.tensor.matmul(
                            ppr[:, half * 512:(half + 1) * 512],
                            pwts[h][:, ot * 128:(ot + 1) * 128],
                            attns[h][:, half * 512:(half + 1) * 512],
                            start=(h == 0), stop=(h == NUM_HEADS - 1))
                o_t = p_out.tile([128, N], f32, tag="out", name=f"o{ot}")
                nc.vector.scalar_tensor_tensor(o_t[:], ppr[:],
                                               beff[:, ot:ot + 1], xs[ot][:],
                                               op0=Alu.add, op1=Alu.add)
                eng = nc.scalar if ot % 2 == 1 else nc.sync
                eng.dma_start(d_out[ot * 128:(ot + 1) * 128, :], o_t[:])

    nc.compile()
    return nc


def make_in_maps(x, norm_w, norm_b, qkv_w, qkv_b, proj_w, proj_b):
    x = np.asarray(x, dtype=np.float32)
    qkv_w = np.asarray(qkv_w, dtype=np.float32)
    qkv_b = np.asarray(qkv_b, dtype=np.float32)
    proj_w = np.asarray(proj_w, dtype=np.float32)
    proj_b = np.asarray(proj_b, dtype=np.float32)

    import ml_dtypes
    wt = np.ascontiguousarray(qkv_w.T).astype(ml_dtypes.bfloat16)   # [C, 3C]
    pwt = np.ascontiguousarray(proj_w.T).astype(ml_dtypes.bfloat16)  # [C, C]
    b_eff = (proj_b + proj_w @ qkv_b[2 * C:3 * C]).astype(np.float32)
    bias_qk = np.ascontiguousarray(qkv_b[:2 * C])

    ch = np.arange(C)
    sel = (ch[:, None] // GROUP_CH == np.arange(NUM_GROUPS)[None, :]).astype(np.float32)
    selT = np.ascontiguousarray(sel.T)

    xs = x.reshape(B, C, N)
    common = {
        "qkv_wt": wt, "proj_wt": pwt, "bias_qk": bias_qk, "b_eff": b_eff,
        "norm_w": np.ascontiguousarray(norm_w, dtype=np.float32),
        "norm_b": np.ascontiguousarray(norm_b, dtype=np.float32),
        "sel": sel, "selT": selT,
        "warm": np.full((128, 512), 0.5, ml_dtypes.bfloat16),
    }
    return [dict(common, x=np.ascontiguousarray(xs[i])) for i in range(B)]


def run(inputs, trace=False, tmpdir=None):
    from concourse.bass_utils import run_bass_kernel_spmd
    nc = build_bass()
    in_maps = make_in_maps(**inputs)
    res = run_bass_kernel_spmd(nc, in_maps, core_ids=list(range(N_CORES)),
                               trace=trace, tmpdir=tmpdir)
    out = np.stack([res.results[i]["out"] for i in range(N_CORES)])
    return out.reshape(B, C, H, W).astype(np.float32), res


def kernel(**inputs):
    out, _ = run(inputs, trace=False)
    return out
